# revision 14
# baseline (speedup 1.0000x reference)
"""LogitLinear Trainium2 kernel: softmax-moment weights + dual fp8 GEMM.

out[n, 0, o] = sum_i mean(W_logits[:, o, i]) * x[n, i]   + mean(b_logits[:, o])
out[n, 1, o] = sum_i var(W_logits[:, o, i])  * x[n, i]^2 + var(b_logits[:, o])

Per (o, i): with u = e^{l0-l1}, v = e^{l2-l1}, r = 1/(1+u+v):
  mean = (v-u)*r,  E[w^2] = (u+v)*r = 1-r,  var = (1-r) - mean^2.
Moment weights, x and x^2 are fp8(e4m3); both GEMMs are DoubleRow fp8
matmuls (K=256 per MM). r comes from a fused custom-DVE op
(bitwise-not seed + 1 Newton step) applied directly to (u, v).

Phase schedule per core (PSUM = 8 banks of [128, 512]):
  M0: mean GEMM n[0:1024)    - overlapped with the moment pipeline
  M1: mean GEMM n[1024:2048) - overlapped with var-weight production
  V0: var GEMM n[0:1024)
  V1: var GEMM n[1024:2048)
Slab loads are split across both HWDGE rings (sync + scalar);
logit-diff loads and output stores ride the gpsimd SWDGE ring.

Sharding: out_feat split across 8 cores (512 each); x replicated.
Host prep: x^T and (x^T)^2 in fp8, logit diffs (l0-l1, l2-l1) in bf16.
"""

import numpy as np
import ml_dtypes

N, IN, OUT, D = 2048, 4096, 4096, 3
NCORES = 8
OS = OUT // NCORES  # 512 out-features per core
KB = IN // 128      # 32 contraction blocks
PAIR = 2            # kb per DoubleRow matmul
KQ = KB // PAIR     # 16 matmul steps over K
CH = 4              # kb per moment-pipeline chunk
NCH = KB // CH      # 8 chunks
NT = N // 128       # 16 n-tiles
MW = 8              # n-tiles per mega-wave (one PSUM bank each)
NMW = NT // MW      # 2 mega-waves per channel
MWS = MW * 128      # 1024 n-columns per mega-wave

RECIP_C0, RECIP_C1 = -0.23549792, 2.0017324

_CACHED_NC = None


def _register_ll_ops():
    """Register fused DVE ops for the moment pipeline (idempotent)."""
    import concourse.dve_ops as dvo
    from concourse.dve_spec import (
        Spec, Src0, Src1, C0, C1, One, Bin, AluOp, lower, _has_src1, sq,
    )
    from concourse.dve_uop import DveOpSpec

    def _recip1(x):
        # bitwise-not exponent-flip seed + Chebyshev scale + 1 Newton pass
        n = Bin(AluOp.BITWISE_NOT, x, x)
        y0 = n * C0
        return y0 * (C1 - x * y0)

    def _np_recip1(x, c0, c1):
        xf = np.ascontiguousarray(x, dtype=np.float32)
        nx = (~xf.view(np.int32)).view(np.float32)
        y0 = nx * c0
        return y0 * (c1 - xf * y0)

    specs = {
        # out = recip1(1 + in0 + in1)
        "LL_RECIPUV": Spec(
            body=_recip1(One + Src0 + Src1),
            reference=lambda in0, in1, s0, s1, imm2: _np_recip1(
                1.0 + np.asarray(in0, np.float32) + np.asarray(in1, np.float32),
                s0, s1,
            ),
        ),
        # out = (1 - in0) - in1^2   [in0=r, in1=mean(fp8)]
        "LL_VAR2": Spec(
            body=(One - Src0) - sq(Src1),
            reference=lambda in0, in1, s0, s1, imm2: (
                (1.0 - np.asarray(in0, np.float32))
                - np.asarray(in1, np.float32) ** 2
            ),
        ),
    }
    ops = {}
    by_name = {op.name: op for op in dvo.OPS}
    for name, spec in specs.items():
        if name in by_name:
            ops[name] = by_name[name]
            continue
        row = dvo._CUSTOM_DVE_ROW_BASE + len(dvo.OPS)
        shas = {}
        for ver in ("v3", "v4"):
            uops = lower(spec, ver=ver)
            shas[ver] = DveOpSpec(
                name=name, opcode=row, uops=uops, rd1_en=_has_src1(spec)
            ).sha(ver)
        op = dvo.DveOp(name, spec, subdim=False, uops_sha=shas)
        dvo.OPS.append(op)
        dvo.CUSTOM_DVE_SPECS[name] = spec
        dvo._SUB_OPCODE_FOR_NAME[name] = row
        ops[name] = op
    return ops


def _build():
    global _CACHED_NC
    if _CACHED_NC is not None:
        return _CACHED_NC
    import concourse.bass as bass
    import concourse.bacc as bacc
    import concourse.mybir as mybir
    import concourse.tile as tile

    ops = _register_ll_ops()
    LL_RECIPUV, LL_VAR2 = ops["LL_RECIPUV"], ops["LL_VAR2"]

    dt = mybir.dt
    f32, bf16, f8 = dt.float32, dt.bfloat16, dt.float8e4
    Exp = mybir.ActivationFunctionType.Exp
    DR = mybir.MatmulPerfMode.DoubleRow

    nc = bacc.Bacc("TRN2", debug=False, num_devices=NCORES)
    xt = nc.dram_tensor("xt", [IN, N], f8, kind="ExternalInput")
    xxt = nc.dram_tensor("xxt", [IN, N], f8, kind="ExternalInput")
    wd = nc.dram_tensor("wd", [2, IN, OS], bf16, kind="ExternalInput")
    bd = nc.dram_tensor("bd", [128, 2, OS], f32, kind="ExternalInput")
    out = nc.dram_tensor("out", [N, 2, OS], f32, kind="ExternalOutput")

    # x^T / (x^T)^2: partition = i within 128-block, free = [kb, n]
    xt_ap = xt.ap().rearrange("(kb p) n -> p kb n", p=128)
    xxt_ap = xxt.ap().rearrange("(kb p) n -> p kb n", p=128)
    # logit diffs per chunk: [p, e, 4, OS]
    wd_ap = wd.ap().rearrange("e (ch p4 p) o -> ch p e p4 o", p=128, p4=CH)
    out_ap = out.ap().rearrange("(nt p) m o -> nt p m o", p=128)

    with tile.TileContext(nc) as tc:
        with (
            tc.tile_pool(name="wres", bufs=1) as wres,
            tc.tile_pool(name="ld", bufs=2) as ld,
            tc.tile_pool(name="mt", bufs=2) as mt,
            tc.tile_pool(name="rr", bufs=NCH + 1) as rrp,
            tc.tile_pool(name="xs", bufs=2) as xs,
            tc.tile_pool(name="st", bufs=4) as st,
            tc.tile_pool(name="bias", bufs=1) as bias,
            tc.tile_pool(name="ps", bufs=8, space="PSUM") as ps,
        ):
            wTm = wres.tile([128, KB, OS], f8, tag="wTm")
            wTv = wres.tile([128, KB, OS], f8, tag="wTv")

            def load_slab(src_ap, mw):
                """x / x^2 column slab, split across both HWDGE rings."""
                sl = xs.tile([128, KB, MWS], f8, tag="xsl")
                half = KB // 2
                cols = slice(mw * MWS, (mw + 1) * MWS)
                nc.sync.dma_start(out=sl[:, :half], in_=src_ap[:, :half, cols])
                nc.scalar.dma_start(out=sl[:, half:], in_=src_ap[:, half:, cols])
                return sl

            # prefetch the M0/M1 x slabs immediately
            xsl0 = load_slab(xt_ap, 0)
            xsl1 = load_slab(xt_ap, 1)

            # warm the ACT exp table before the first real exp
            warm = wres.tile([1, 8], f32, tag="warm")
            nc.vector.memset(warm, 0.0)
            nc.scalar.activation(out=warm, in_=warm, func=Exp)

            def emit_bias():
                bdt = bias.tile([128, 2, OS], f32, tag="bdt")
                nc.gpsimd.dma_start(out=bdt, in_=bd.ap())
                bee = bias.tile([128, 2, OS], bf16, tag="bee")
                nc.scalar.activation(out=bee, in_=bdt, func=Exp)
                ba = bias.tile([128, OS], bf16, tag="ba")
                nc.gpsimd.tensor_sub(ba, bee[:, 1], bee[:, 0])
                br = bias.tile([128, OS], bf16, tag="br")
                nc.vector._custom_dve(
                    LL_RECIPUV, out=br, in0=bee[:, 0], in1=bee[:, 1],
                    s0=RECIP_C0, s1=RECIP_C1,
                )
                bmean = bias.tile([128, OS], bf16, tag="bmean")
                nc.gpsimd.tensor_mul(bmean, ba, br)
                bvar = bias.tile([128, OS], bf16, tag="bvar")
                nc.vector._custom_dve(LL_VAR2, out=bvar, in0=br, in1=bmean)
                return bmean, bvar

            def emit_moments(ch):
                lt = ld.tile([128, 2, CH, OS], bf16, tag="lt")
                for e in range(2):
                    nc.gpsimd.dma_start(out=lt[:, e], in_=wd_ap[ch][:, e])
                ee = mt.tile([128, 2, CH, OS], bf16, tag="ee")
                nc.scalar.activation(out=ee, in_=lt, func=Exp)
                a = mt.tile([128, CH, OS], bf16, tag="a")
                nc.vector.tensor_sub(a, ee[:, 1], ee[:, 0])
                r = rrp.tile([128, CH, OS], bf16, tag="r")
                nc.vector._custom_dve(
                    LL_RECIPUV, out=r, in0=ee[:, 0], in1=ee[:, 1],
                    s0=RECIP_C0, s1=RECIP_C1,
                )
                nc.vector.tensor_mul(wTm[:, CH * ch : CH * (ch + 1), :], a, r)
                return r

            def emit_var_weights(ch, r):
                nc.vector._custom_dve(
                    LL_VAR2,
                    out=wTv[:, CH * ch : CH * (ch + 1), :],
                    in0=r,
                    in1=wTm[:, CH * ch : CH * (ch + 1), :],
                )

            def mm(bank, slab, kq, j, w_t, start, stop):
                nc.tensor.matmul(
                    bank,
                    lhsT=slab[:, PAIR * kq : PAIR * (kq + 1),
                              j * 128 : (j + 1) * 128],
                    rhs=w_t[:, PAIR * kq : PAIR * (kq + 1), :],
                    start=start,
                    stop=stop,
                    perf_mode=DR,
                )

            def evac(mw, j, m, bank, b_t):
                stg = st.tile([128, OS], f32, tag="stg")
                nc.vector.tensor_add(stg, bank, b_t)
                nc.gpsimd.dma_start(out=out_ap[mw * MW + j][:, m, :], in_=stg)

            r_t = [None] * NCH

            # ---- M0: mean n[0:1024) + moment pipeline -----------------------
            pb = mean = [
                ps.tile([128, OS], f32, tag="ps", name=f"psm0_{j}")
                for j in range(MW)
            ]
            bmean = bvar = None
            for ch in range(NCH):
                r_t[ch] = emit_moments(ch)
                if ch == 0:
                    bmean, bvar = emit_bias()
                for kq in (2 * ch, 2 * ch + 1):
                    for j in range(MW):
                        mm(pb[j], xsl0, kq, j, wTm, kq == 0, kq == KQ - 1)
            for j in range(MW):
                evac(0, j, 0, pb[j], bmean)

            # ---- M1: mean n[1024:2048) + var weights -------------------------
            xxl0 = load_slab(xxt_ap, 0)  # prefetch V0 slab (reuses M0 buffer)
            pb = [
                ps.tile([128, OS], f32, tag="ps", name=f"psm1_{j}")
                for j in range(MW)
            ]
            for j in range(MW):
                for kq in range(KQ):
                    mm(pb[j], xsl1, kq, j, wTm, kq == 0, kq == KQ - 1)
                emit_var_weights(j, r_t[j])
                r_t[j] = None
                evac(1, j, 0, pb[j], bmean)

            # ---- V0 / V1: var GEMMs ------------------------------------------
            for mw in range(NMW):
                xxl = xxl0 if mw == 0 else load_slab(xxt_ap, 1)
                pv = [
                    ps.tile([128, OS], f32, tag="ps", name=f"psv{mw}_{j}")
                    for j in range(MW)
                ]
                for j in range(MW):
                    for kq in range(KQ):
                        mm(pv[j], xxl, kq, j, wTv, kq == 0, kq == KQ - 1)
                    evac(mw, j, 1, pv[j], bvar)

    nc.compile()
    _CACHED_NC = nc
    return nc


def _prep_inputs(x, W_logits, b_logits):
    f8np = ml_dtypes.float8_e4m3
    bf16np = ml_dtypes.bfloat16
    xt_8 = np.ascontiguousarray(x.T).astype(f8np)
    xxt_8 = (xt_8.astype(np.float32) ** 2).astype(f8np)
    # logit diffs (softmax is shift invariant): l0-l1, l2-l1
    wdiff = np.stack([W_logits[0] - W_logits[1], W_logits[2] - W_logits[1]])
    bdiff = np.stack(
        [b_logits[0, :, 0] - b_logits[1, :, 0], b_logits[2, :, 0] - b_logits[1, :, 0]]
    ).astype(np.float32)
    in_maps = []
    for c in range(NCORES):
        sl = slice(c * OS, (c + 1) * OS)
        wd_c = np.ascontiguousarray(
            wdiff[:, sl, :].transpose(0, 2, 1)
        ).astype(bf16np)
        bd_c = np.ascontiguousarray(
            np.broadcast_to(bdiff[None, :, sl], (128, 2, OS))
        )
        in_maps.append({"xt": xt_8, "xxt": xxt_8, "wd": wd_c, "bd": bd_c})
    return in_maps


def kernel(x, W_logits, b_logits):
    from concourse import bass_utils

    nc = _build()
    in_maps = _prep_inputs(x, W_logits, b_logits)
    res = bass_utils.run_bass_kernel_spmd(
        nc, in_maps, core_ids=list(range(NCORES))
    )
    full = np.empty((N, 2, OUT), dtype=np.float32)
    for c in range(NCORES):
        full[:, :, c * OS : (c + 1) * OS] = res.results[c]["out"]
    return full


# revision 20
# speedup vs baseline: 1.1715x; 1.1715x over previous
"""LogitLinear Trainium2 kernel: softmax-moment weights + dual fp8 GEMM.

out[n, 0, o] = sum_i mean(W_logits[:, o, i]) * x[n, i]   + mean(b_logits[:, o])
out[n, 1, o] = sum_i var(W_logits[:, o, i])  * x[n, i]^2 + var(b_logits[:, o])

Per (o, i): with u = e^{l0-l1}, v = e^{l2-l1}, r = 1/(1+u+v):
  mean = (v-u)*r,  E[w^2] = (u+v)*r = 1-r,  var = (1-r) - mean^2.
Moment weights, x and x^2 are fp8(e4m3); both GEMMs are DoubleRow fp8
matmuls (K=256 per MM). r comes from a fused custom-DVE op
(bitwise-not seed + 1 Newton step) applied directly to (u, v).

Phase schedule per core (PSUM = 8 banks of [128, 512]):
  M0: mean GEMM n[0:1024)    - overlapped with the moment pipeline
  M1: mean GEMM n[1024:2048) - overlapped with var-weight production
  V0: var GEMM n[0:1024)
  V1: var GEMM n[1024:2048)
x / x^2 ship as four [IN, 1024] column slabs so every DMA row is
partition-contiguous (long coalesced descriptors); slab loads and
output stores alternate between the two HWDGE rings (sync + scalar);
logit-diff loads ride the gpsimd SWDGE ring.

Sharding: out_feat split across 8 cores (512 each); x replicated.
Host prep: x^T / (x^T)^2 slabs in fp8, logit diffs (l0-l1, l2-l1) bf16.
"""

import numpy as np
import ml_dtypes

N, IN, OUT, D = 2048, 4096, 4096, 3
NCORES = 8
OS = OUT // NCORES  # 512 out-features per core
KB = IN // 128      # 32 contraction blocks
PAIR = 2            # kb per DoubleRow matmul
KQ = KB // PAIR     # 16 matmul steps over K
CH = 4              # kb per moment-pipeline chunk
NCH = KB // CH      # 8 chunks
NT = N // 128       # 16 n-tiles
MW = 8              # n-tiles per mega-wave (one PSUM bank each)
NMW = NT // MW      # 2 mega-waves per channel
MWS = MW * 128      # 1024 n-columns per mega-wave

RECIP_C0, RECIP_C1 = -0.23549792, 2.0017324

_CACHED_NC = None


def _register_ll_ops():
    """Register fused DVE ops for the moment pipeline (idempotent)."""
    import concourse.dve_ops as dvo
    from concourse.dve_spec import (
        Spec, Src0, Src1, C0, C1, One, Bin, AluOp, lower, _has_src1, sq,
    )
    from concourse.dve_uop import DveOpSpec

    def _recip1(x):
        # bitwise-not exponent-flip seed + Chebyshev scale + 1 Newton pass
        n = Bin(AluOp.BITWISE_NOT, x, x)
        y0 = n * C0
        return y0 * (C1 - x * y0)

    def _np_recip1(x, c0, c1):
        xf = np.ascontiguousarray(x, dtype=np.float32)
        nx = (~xf.view(np.int32)).view(np.float32)
        y0 = nx * c0
        return y0 * (c1 - xf * y0)

    specs = {
        # out = recip1(1 + in0 + in1)
        "LL_RECIPUV": Spec(
            body=_recip1(One + Src0 + Src1),
            reference=lambda in0, in1, s0, s1, imm2: _np_recip1(
                1.0 + np.asarray(in0, np.float32) + np.asarray(in1, np.float32),
                s0, s1,
            ),
        ),
        # out = (1 - in0) - in1^2   [in0=r, in1=mean(fp8)]
        "LL_VAR2": Spec(
            body=(One - Src0) - sq(Src1),
            reference=lambda in0, in1, s0, s1, imm2: (
                (1.0 - np.asarray(in0, np.float32))
                - np.asarray(in1, np.float32) ** 2
            ),
        ),
    }
    ops = {}
    by_name = {op.name: op for op in dvo.OPS}
    for name, spec in specs.items():
        if name in by_name:
            ops[name] = by_name[name]
            continue
        row = dvo._CUSTOM_DVE_ROW_BASE + len(dvo.OPS)
        shas = {}
        for ver in ("v3", "v4"):
            uops = lower(spec, ver=ver)
            shas[ver] = DveOpSpec(
                name=name, opcode=row, uops=uops, rd1_en=_has_src1(spec)
            ).sha(ver)
        op = dvo.DveOp(name, spec, subdim=False, uops_sha=shas)
        dvo.OPS.append(op)
        dvo.CUSTOM_DVE_SPECS[name] = spec
        dvo._SUB_OPCODE_FOR_NAME[name] = row
        ops[name] = op
    return ops


def _build():
    global _CACHED_NC
    if _CACHED_NC is not None:
        return _CACHED_NC
    import concourse.bass as bass
    import concourse.bacc as bacc
    import concourse.mybir as mybir
    import concourse.tile as tile

    ops = _register_ll_ops()
    LL_RECIPUV, LL_VAR2 = ops["LL_RECIPUV"], ops["LL_VAR2"]

    dt = mybir.dt
    f32, bf16, f8 = dt.float32, dt.bfloat16, dt.float8e4
    Exp = mybir.ActivationFunctionType.Exp
    DR = mybir.MatmulPerfMode.DoubleRow

    nc = bacc.Bacc("TRN2", debug=False, num_devices=NCORES)
    xs_dram = [
        nc.dram_tensor(nm, [IN, MWS], f8, kind="ExternalInput")
        for nm in ("xt0", "xt1", "xxt0", "xxt1")
    ]
    wd = nc.dram_tensor("wd", [2, IN, OS], bf16, kind="ExternalInput")
    bd = nc.dram_tensor("bd", [128, 2, OS], f32, kind="ExternalInput")
    out = nc.dram_tensor("out", [N, 2, OS], f32, kind="ExternalOutput")

    # slabs: partition = i within 128-block, free = [kb, n]
    slab_aps = [
        t.ap().rearrange("(kb p) n -> p kb n", p=128) for t in xs_dram
    ]
    # logit diffs per chunk: [p, e, 4, OS]
    wd_ap = wd.ap().rearrange("e (ch p4 p) o -> ch p e p4 o", p=128, p4=CH)
    out_ap = out.ap().rearrange("(nt p) m o -> nt p m o", p=128)

    with tile.TileContext(nc) as tc:
        with (
            tc.tile_pool(name="wres", bufs=1) as wres,
            tc.tile_pool(name="ld", bufs=2) as ld,
            tc.tile_pool(name="mt", bufs=2) as mt,
            tc.tile_pool(name="rr", bufs=NCH + 1) as rrp,
            tc.tile_pool(name="xs", bufs=2) as xs,
            tc.tile_pool(name="st", bufs=4) as st,
            tc.tile_pool(name="bias", bufs=1) as bias,
            tc.tile_pool(name="ps", bufs=8, space="PSUM") as ps,
        ):
            wTm = wres.tile([128, KB, OS], f8, tag="wTm")
            wTv = wres.tile([128, KB, OS], f8, tag="wTv")

            def load_slab(idx, eng):
                sl = xs.tile([128, KB, MWS], f8, tag="xsl")
                eng.dma_start(out=sl, in_=slab_aps[idx])
                return sl

            # x slabs for M0/M1, one per HWDGE ring
            xsl0 = load_slab(0, nc.sync)
            xsl1 = load_slab(1, nc.scalar)

            # warm the ACT exp table before the first real exp
            warm = wres.tile([1, 8], f32, tag="warm")
            nc.vector.memset(warm, 0.0)
            nc.scalar.activation(out=warm, in_=warm, func=Exp)

            def emit_bias():
                bdt = bias.tile([128, 2, OS], f32, tag="bdt")
                nc.gpsimd.dma_start(out=bdt, in_=bd.ap())
                bee = bias.tile([128, 2, OS], bf16, tag="bee")
                nc.scalar.activation(out=bee, in_=bdt, func=Exp)
                ba = bias.tile([128, OS], bf16, tag="ba")
                nc.gpsimd.tensor_sub(ba, bee[:, 1], bee[:, 0])
                br = bias.tile([128, OS], bf16, tag="br")
                nc.vector._custom_dve(
                    LL_RECIPUV, out=br, in0=bee[:, 0], in1=bee[:, 1],
                    s0=RECIP_C0, s1=RECIP_C1,
                )
                bmean = bias.tile([128, OS], bf16, tag="bmean")
                nc.gpsimd.tensor_mul(bmean, ba, br)
                bvar = bias.tile([128, OS], bf16, tag="bvar")
                nc.vector._custom_dve(LL_VAR2, out=bvar, in0=br, in1=bmean)
                return bmean, bvar

            def emit_moments(ch):
                lt = ld.tile([128, 2, CH, OS], bf16, tag="lt")
                for e in range(2):
                    nc.gpsimd.dma_start(out=lt[:, e], in_=wd_ap[ch][:, e])
                ee = mt.tile([128, 2, CH, OS], bf16, tag="ee")
                nc.scalar.activation(out=ee, in_=lt, func=Exp)
                a = mt.tile([128, CH, OS], bf16, tag="a")
                nc.vector.tensor_sub(a, ee[:, 1], ee[:, 0])
                r = rrp.tile([128, CH, OS], f8, tag="r")
                nc.vector._custom_dve(
                    LL_RECIPUV, out=r, in0=ee[:, 0], in1=ee[:, 1],
                    s0=RECIP_C0, s1=RECIP_C1,
                )
                nc.vector.tensor_mul(wTm[:, CH * ch : CH * (ch + 1), :], a, r)
                return r

            def emit_var_weights(ch, r):
                nc.vector._custom_dve(
                    LL_VAR2,
                    out=wTv[:, CH * ch : CH * (ch + 1), :],
                    in0=r,
                    in1=wTm[:, CH * ch : CH * (ch + 1), :],
                )

            def mm(bank, slab, kq, j, w_t, start, stop):
                nc.tensor.matmul(
                    bank,
                    lhsT=slab[:, PAIR * kq : PAIR * (kq + 1),
                              j * 128 : (j + 1) * 128],
                    rhs=w_t[:, PAIR * kq : PAIR * (kq + 1), :],
                    start=start,
                    stop=stop,
                    perf_mode=DR,
                )

            def evac(mw, j, m, bank, b_t):
                stg = st.tile([128, OS], f32, tag="stg")
                nc.vector.tensor_add(stg, bank, b_t)
                eng = nc.sync if j % 2 == 0 else nc.scalar
                eng.dma_start(out=out_ap[mw * MW + j][:, m, :], in_=stg)

            r_t = [None] * NCH

            # ---- M0: mean n[0:1024) + moment pipeline -----------------------
            pb = [
                ps.tile([128, OS], f32, tag="ps", name=f"psm0_{j}")
                for j in range(MW)
            ]
            bmean = bvar = None
            for ch in range(NCH):
                r_t[ch] = emit_moments(ch)
                if ch == 0:
                    bmean, bvar = emit_bias()
                for kq in (2 * ch, 2 * ch + 1):
                    for j in range(MW):
                        mm(pb[j], xsl0, kq, j, wTm, kq == 0, kq == KQ - 1)
            for j in range(MW):
                evac(0, j, 0, pb[j], bmean)

            # ---- M1: mean n[1024:2048) + var weights -------------------------
            xxl0 = load_slab(2, nc.sync)  # prefetch V0 slab (reuses M0 buffer)
            pb = [
                ps.tile([128, OS], f32, tag="ps", name=f"psm1_{j}")
                for j in range(MW)
            ]
            for j in range(MW):
                for kq in range(KQ):
                    mm(pb[j], xsl1, kq, j, wTm, kq == 0, kq == KQ - 1)
                emit_var_weights(j, r_t[j])
                r_t[j] = None
                evac(1, j, 0, pb[j], bmean)

            # ---- V0 / V1: var GEMMs ------------------------------------------
            xxl1 = load_slab(3, nc.scalar)  # prefetch V1 slab
            for mw in range(NMW):
                xxl = xxl0 if mw == 0 else xxl1
                pv = [
                    ps.tile([128, OS], f32, tag="ps", name=f"psv{mw}_{j}")
                    for j in range(MW)
                ]
                for j in range(MW):
                    for kq in range(KQ):
                        mm(pv[j], xxl, kq, j, wTv, kq == 0, kq == KQ - 1)
                    evac(mw, j, 1, pv[j], bvar)

    nc.compile()
    _CACHED_NC = nc
    return nc


def _prep_inputs(x, W_logits, b_logits):
    f8np = ml_dtypes.float8_e4m3
    bf16np = ml_dtypes.bfloat16
    xt_8 = np.ascontiguousarray(x.T).astype(f8np)
    xxt_8 = (xt_8.astype(np.float32) ** 2).astype(f8np)
    slabs = {
        "xt0": np.ascontiguousarray(xt_8[:, :MWS]),
        "xt1": np.ascontiguousarray(xt_8[:, MWS:]),
        "xxt0": np.ascontiguousarray(xxt_8[:, :MWS]),
        "xxt1": np.ascontiguousarray(xxt_8[:, MWS:]),
    }
    # logit diffs (softmax is shift invariant): l0-l1, l2-l1
    wdiff = np.stack([W_logits[0] - W_logits[1], W_logits[2] - W_logits[1]])
    bdiff = np.stack(
        [b_logits[0, :, 0] - b_logits[1, :, 0], b_logits[2, :, 0] - b_logits[1, :, 0]]
    ).astype(np.float32)
    in_maps = []
    for c in range(NCORES):
        sl = slice(c * OS, (c + 1) * OS)
        wd_c = np.ascontiguousarray(
            wdiff[:, sl, :].transpose(0, 2, 1)
        ).astype(bf16np)
        bd_c = np.ascontiguousarray(
            np.broadcast_to(bdiff[None, :, sl], (128, 2, OS))
        )
        in_maps.append({**slabs, "wd": wd_c, "bd": bd_c})
    return in_maps


def kernel(x, W_logits, b_logits):
    from concourse import bass_utils

    nc = _build()
    in_maps = _prep_inputs(x, W_logits, b_logits)
    res = bass_utils.run_bass_kernel_spmd(
        nc, in_maps, core_ids=list(range(NCORES))
    )
    full = np.empty((N, 2, OUT), dtype=np.float32)
    for c in range(NCORES):
        full[:, :, c * OS : (c + 1) * OS] = res.results[c]["out"]
    return full


# revision 24
# speedup vs baseline: 1.1808x; 1.0079x over previous
"""LogitLinear Trainium2 kernel: softmax-moment weights + dual fp8 GEMM.

out[n, 0, o] = sum_i mean(W_logits[:, o, i]) * x[n, i]   + mean(b_logits[:, o])
out[n, 1, o] = sum_i var(W_logits[:, o, i])  * x[n, i]^2 + var(b_logits[:, o])

Per (o, i): with u = e^{l0-l1}, v = e^{l2-l1}, r = 1/(1+u+v):
  mean = (v-u)*r,  E[w^2] = (u+v)*r = 1-r,  var = (1-r) - mean^2.
Moment weights, x and x^2 are fp8(e4m3); both GEMMs are DoubleRow fp8
matmuls (K=256 per MM). r comes from a fused custom-DVE op
(bitwise-not seed + 1 Newton step) applied directly to (u, v).

Phase schedule per core (PSUM = 8 banks of [128, 512]):
  M0: mean GEMM n[0:1024)    - overlapped with the moment pipeline
  M1: mean GEMM n[1024:2048) - overlapped with var-weight production
  V0: var GEMM n[0:1024)
  V1: var GEMM n[1024:2048)
x / x^2 ship as four [IN, 1024] column slabs so every DMA row is
partition-contiguous (long coalesced descriptors); slab loads and
output stores alternate between the two HWDGE rings (sync + scalar);
logit-diff loads ride the gpsimd SWDGE ring.

Sharding: out_feat split across 8 cores (512 each); x replicated.
Host prep: x^T / (x^T)^2 slabs in fp8, logit diffs (l0-l1, l2-l1) bf16.
"""

import numpy as np
import ml_dtypes

N, IN, OUT, D = 2048, 4096, 4096, 3
NCORES = 8
OS = OUT // NCORES  # 512 out-features per core
KB = IN // 128      # 32 contraction blocks
PAIR = 2            # kb per DoubleRow matmul
KQ = KB // PAIR     # 16 matmul steps over K
CH = 4              # kb per moment-pipeline chunk
NCH = KB // CH      # 8 chunks
NT = N // 128       # 16 n-tiles
MW = 8              # n-tiles per mega-wave (one PSUM bank each)
NMW = NT // MW      # 2 mega-waves per channel
MWS = MW * 128      # 1024 n-columns per mega-wave

RECIP_C0, RECIP_C1 = -0.23549792, 2.0017324

_CACHED_NC = None


def _register_ll_ops():
    """Register fused DVE ops for the moment pipeline (idempotent)."""
    import concourse.dve_ops as dvo
    from concourse.dve_spec import (
        Spec, Src0, Src1, C0, C1, One, Bin, AluOp, lower, _has_src1, sq,
    )
    from concourse.dve_uop import DveOpSpec

    def _recip1(x):
        # bitwise-not exponent-flip seed + Chebyshev scale + 1 Newton pass
        n = Bin(AluOp.BITWISE_NOT, x, x)
        y0 = n * C0
        return y0 * (C1 - x * y0)

    def _np_recip1(x, c0, c1):
        xf = np.ascontiguousarray(x, dtype=np.float32)
        nx = (~xf.view(np.int32)).view(np.float32)
        y0 = nx * c0
        return y0 * (c1 - xf * y0)

    specs = {
        # out = recip1(1 + in0 + in1)
        "LL_RECIPUV": Spec(
            body=_recip1(One + Src0 + Src1),
            reference=lambda in0, in1, s0, s1, imm2: _np_recip1(
                1.0 + np.asarray(in0, np.float32) + np.asarray(in1, np.float32),
                s0, s1,
            ),
        ),
        # out = (1 - in0) - in1^2   [in0=r, in1=mean(fp8)]
        "LL_VAR2": Spec(
            body=(One - Src0) - sq(Src1),
            reference=lambda in0, in1, s0, s1, imm2: (
                (1.0 - np.asarray(in0, np.float32))
                - np.asarray(in1, np.float32) ** 2
            ),
        ),
    }
    ops = {}
    by_name = {op.name: op for op in dvo.OPS}
    for name, spec in specs.items():
        if name in by_name:
            ops[name] = by_name[name]
            continue
        row = dvo._CUSTOM_DVE_ROW_BASE + len(dvo.OPS)
        shas = {}
        for ver in ("v3", "v4"):
            uops = lower(spec, ver=ver)
            shas[ver] = DveOpSpec(
                name=name, opcode=row, uops=uops, rd1_en=_has_src1(spec)
            ).sha(ver)
        op = dvo.DveOp(name, spec, subdim=False, uops_sha=shas)
        dvo.OPS.append(op)
        dvo.CUSTOM_DVE_SPECS[name] = spec
        dvo._SUB_OPCODE_FOR_NAME[name] = row
        ops[name] = op
    return ops


def _build():
    global _CACHED_NC
    if _CACHED_NC is not None:
        return _CACHED_NC
    import concourse.bass as bass
    import concourse.bacc as bacc
    import concourse.mybir as mybir
    import concourse.tile as tile

    ops = _register_ll_ops()
    LL_RECIPUV, LL_VAR2 = ops["LL_RECIPUV"], ops["LL_VAR2"]

    dt = mybir.dt
    f32, bf16, f8 = dt.float32, dt.bfloat16, dt.float8e4
    Exp = mybir.ActivationFunctionType.Exp
    DR = mybir.MatmulPerfMode.DoubleRow

    nc = bacc.Bacc("TRN2", debug=False, num_devices=NCORES)
    xs_dram = [
        nc.dram_tensor(nm, [IN, MWS], f8, kind="ExternalInput")
        for nm in ("xt0", "xt1", "xxt0", "xxt1")
    ]
    wd = nc.dram_tensor("wd", [2, IN, OS], bf16, kind="ExternalInput")
    bd = nc.dram_tensor("bd", [128, 2, OS], f32, kind="ExternalInput")
    out = nc.dram_tensor("out", [N, 2, OS], f32, kind="ExternalOutput")

    # slabs: partition = i within 128-block, free = [kb, n]
    slab_aps = [
        t.ap().rearrange("(kb p) n -> p kb n", p=128) for t in xs_dram
    ]
    # logit diffs per chunk: [p, e, 4, OS]
    wd_ap = wd.ap().rearrange("e (ch p4 p) o -> ch p e p4 o", p=128, p4=CH)
    out_ap = out.ap().rearrange("(nt p) m o -> nt p m o", p=128)

    with tile.TileContext(nc) as tc:
        with (
            tc.tile_pool(name="wres", bufs=1) as wres,
            tc.tile_pool(name="ld", bufs=2) as ld,
            tc.tile_pool(name="mt", bufs=2) as mt,
            tc.tile_pool(name="rr", bufs=NCH + 1) as rrp,
            tc.tile_pool(name="xs", bufs=2) as xs,
            tc.tile_pool(name="st", bufs=4) as st,
            tc.tile_pool(name="bias", bufs=1) as bias,
            tc.tile_pool(name="ps", bufs=8, space="PSUM") as ps,
        ):
            wTm = wres.tile([128, KB, OS], f8, tag="wTm")
            wTv = wres.tile([128, KB, OS], f8, tag="wTv")

            def load_slab(idx, split=2):
                """Load a slab in kb-group pieces alternating HWDGE rings."""
                sl = xs.tile([128, KB, MWS], f8, tag="xsl")
                g = KB // split
                for i in range(split):
                    eng = nc.sync if i % 2 == 0 else nc.scalar
                    eng.dma_start(
                        out=sl[:, i * g : (i + 1) * g],
                        in_=slab_aps[idx][:, i * g : (i + 1) * g],
                    )
                return sl

            # M0 x slab: fine-grained so the first matmuls start early.
            # kq consumes kb pairs in order, so early groups unblock early.
            xsl0 = load_slab(0, split=4)

            # warm the ACT exp table before the first real exp
            warm = wres.tile([1, 8], f32, tag="warm")
            nc.vector.memset(warm, 0.0)
            nc.scalar.activation(out=warm, in_=warm, func=Exp)

            def emit_bias():
                bdt = bias.tile([128, 2, OS], f32, tag="bdt")
                nc.gpsimd.dma_start(out=bdt, in_=bd.ap())
                bee = bias.tile([128, 2, OS], bf16, tag="bee")
                nc.scalar.activation(out=bee, in_=bdt, func=Exp)
                ba = bias.tile([128, OS], bf16, tag="ba")
                nc.gpsimd.tensor_sub(ba, bee[:, 1], bee[:, 0])
                br = bias.tile([128, OS], bf16, tag="br")
                nc.vector._custom_dve(
                    LL_RECIPUV, out=br, in0=bee[:, 0], in1=bee[:, 1],
                    s0=RECIP_C0, s1=RECIP_C1,
                )
                bmean = bias.tile([128, OS], bf16, tag="bmean")
                nc.gpsimd.tensor_mul(bmean, ba, br)
                bvar = bias.tile([128, OS], bf16, tag="bvar")
                nc.vector._custom_dve(LL_VAR2, out=bvar, in0=br, in1=bmean)
                return bmean, bvar

            def emit_moments(ch):
                lt = ld.tile([128, 2, CH, OS], bf16, tag="lt")
                for e in range(2):
                    nc.gpsimd.dma_start(out=lt[:, e], in_=wd_ap[ch][:, e])
                ee = mt.tile([128, 2, CH, OS], bf16, tag="ee")
                nc.scalar.activation(out=ee, in_=lt, func=Exp)
                a = mt.tile([128, CH, OS], bf16, tag="a")
                nc.vector.tensor_sub(a, ee[:, 1], ee[:, 0])
                r = rrp.tile([128, CH, OS], f8, tag="r")
                nc.vector._custom_dve(
                    LL_RECIPUV, out=r, in0=ee[:, 0], in1=ee[:, 1],
                    s0=RECIP_C0, s1=RECIP_C1,
                )
                nc.vector.tensor_mul(wTm[:, CH * ch : CH * (ch + 1), :], a, r)
                return r

            def emit_var_weights(ch, r):
                nc.vector._custom_dve(
                    LL_VAR2,
                    out=wTv[:, CH * ch : CH * (ch + 1), :],
                    in0=r,
                    in1=wTm[:, CH * ch : CH * (ch + 1), :],
                )

            def mm(bank, slab, kq, j, w_t, start, stop):
                nc.tensor.matmul(
                    bank,
                    lhsT=slab[:, PAIR * kq : PAIR * (kq + 1),
                              j * 128 : (j + 1) * 128],
                    rhs=w_t[:, PAIR * kq : PAIR * (kq + 1), :],
                    start=start,
                    stop=stop,
                    perf_mode=DR,
                )

            def evac(mw, j, m, bank, b_t):
                stg = st.tile([128, OS], f32, tag="stg")
                nc.vector.tensor_add(stg, bank, b_t)
                eng = (nc.sync, nc.scalar, nc.gpsimd)[j % 3]
                eng.dma_start(out=out_ap[mw * MW + j][:, m, :], in_=stg)

            r_t = [None] * NCH

            # ---- M0: mean n[0:1024) + moment pipeline -----------------------
            pb = [
                ps.tile([128, OS], f32, tag="ps", name=f"psm0_{j}")
                for j in range(MW)
            ]
            bmean = bvar = None
            for ch in range(NCH):
                r_t[ch] = emit_moments(ch)
                if ch == 0:
                    bmean, bvar = emit_bias()
                for kq in (2 * ch, 2 * ch + 1):
                    for j in range(MW):
                        mm(pb[j], xsl0, kq, j, wTm, kq == 0, kq == KQ - 1)
            xsl1 = load_slab(1)  # M1 x slab (starts once the rings free up)
            for j in range(MW):
                evac(0, j, 0, pb[j], bmean)

            # ---- M1: mean n[1024:2048) + var weights -------------------------
            xxl0 = load_slab(2)  # prefetch V0 slab (reuses M0 buffer)
            pb = [
                ps.tile([128, OS], f32, tag="ps", name=f"psm1_{j}")
                for j in range(MW)
            ]
            for j in range(MW):
                for kq in range(KQ):
                    mm(pb[j], xsl1, kq, j, wTm, kq == 0, kq == KQ - 1)
                emit_var_weights(j, r_t[j])
                r_t[j] = None
                evac(1, j, 0, pb[j], bmean)

            # ---- V0 / V1: var GEMMs ------------------------------------------
            xxl1 = load_slab(3)  # prefetch V1 slab
            for mw in range(NMW):
                xxl = xxl0 if mw == 0 else xxl1
                pv = [
                    ps.tile([128, OS], f32, tag="ps", name=f"psv{mw}_{j}")
                    for j in range(MW)
                ]
                for j in range(MW):
                    for kq in range(KQ):
                        mm(pv[j], xxl, kq, j, wTv, kq == 0, kq == KQ - 1)
                    evac(mw, j, 1, pv[j], bvar)

    nc.compile()
    _CACHED_NC = nc
    return nc


def _prep_inputs(x, W_logits, b_logits):
    f8np = ml_dtypes.float8_e4m3
    bf16np = ml_dtypes.bfloat16
    xt_8 = np.ascontiguousarray(x.T).astype(f8np)
    xxt_8 = (xt_8.astype(np.float32) ** 2).astype(f8np)
    slabs = {
        "xt0": np.ascontiguousarray(xt_8[:, :MWS]),
        "xt1": np.ascontiguousarray(xt_8[:, MWS:]),
        "xxt0": np.ascontiguousarray(xxt_8[:, :MWS]),
        "xxt1": np.ascontiguousarray(xxt_8[:, MWS:]),
    }
    # logit diffs (softmax is shift invariant): l0-l1, l2-l1
    wdiff = np.stack([W_logits[0] - W_logits[1], W_logits[2] - W_logits[1]])
    bdiff = np.stack(
        [b_logits[0, :, 0] - b_logits[1, :, 0], b_logits[2, :, 0] - b_logits[1, :, 0]]
    ).astype(np.float32)
    in_maps = []
    for c in range(NCORES):
        sl = slice(c * OS, (c + 1) * OS)
        wd_c = np.ascontiguousarray(
            wdiff[:, sl, :].transpose(0, 2, 1)
        ).astype(bf16np)
        bd_c = np.ascontiguousarray(
            np.broadcast_to(bdiff[None, :, sl], (128, 2, OS))
        )
        in_maps.append({**slabs, "wd": wd_c, "bd": bd_c})
    return in_maps


def kernel(x, W_logits, b_logits):
    from concourse import bass_utils

    nc = _build()
    in_maps = _prep_inputs(x, W_logits, b_logits)
    res = bass_utils.run_bass_kernel_spmd(
        nc, in_maps, core_ids=list(range(NCORES))
    )
    full = np.empty((N, 2, OUT), dtype=np.float32)
    for c in range(NCORES):
        full[:, :, c * OS : (c + 1) * OS] = res.results[c]["out"]
    return full
